# revision 1
# baseline (speedup 1.0000x reference)
"""Correlation kernel v4: quad-batched PSUM evacuation to keep the PE warm.

Same matmul structure as v2/v3 (16x8 stationary blocks, 24x16 halo, d2
SBUF-resident).  The critical fix vs v2: PSUM tiles hold FOUR blocks (4 banks,
[128, 4, 512] fp32, pool bufs=2 = all 8 banks) and each DVE/ACT copy
evacuates all four at once (FD=1536), so copy throughput (~216 ns/block
alternating engines) exceeds the warm PE's 320 ns/block and the tensor engine
never re-throttles.  Bands ship full-width (block-major layout, one ~2 MB DMA
per by-row, alternating sync/scalar HWDGE rings); the diagonal gather stays on
host.
"""

import numpy as np

C, H, W = 256, 96, 160
PAD = 4
NG = 9
Q = NG * NG
GB, BB = 16, 8
TT, UU = GB + 2 * PAD, BB + 2 * PAD  # 24 x 16
NMOV = TT * UU  # 384
NBY, NBX = H // GB, W // BB  # 6 x 20
HP, WP = H + 2 * PAD, W + 2 * PAD
NQUAD = NBX // 4  # 5 quads per by-row
N_CORES = 8

D2_SLABS = [(0, 24)] + [(24 + 16 * i, 40 + 16 * i) for i in range(NBY - 1)]

_CACHE = {}


def _build_bass(reps=1):
    import contextlib

    import concourse.bass as bass  # noqa: F401
    import concourse.mybir as mybir
    import concourse.tile as tile
    from concourse import bacc

    fp16 = mybir.dt.float16
    fp32 = mybir.dt.float32

    nc = bacc.Bacc("TRN2", target_bir_lowering=False, debug=False)

    d1b = nc.dram_tensor("d1b", [C, NBY, NBX, 128], fp16, kind="ExternalInput").ap()
    d2b = nc.dram_tensor("d2b", [C, HP, WP], fp16, kind="ExternalInput").ap()
    bands = nc.dram_tensor(
        "bands", [NBY, 128, NBX, NMOV], fp16, kind="ExternalOutput"
    ).ap()

    d1r = d1b.rearrange("(k p) by bx m -> p k by bx m", p=128)
    d2r = d2b.rearrange("(k p) y x -> p k y x", p=128)

    with tile.TileContext(nc) as tc:
        with (
            tc.tile_pool(name="in2", bufs=1) as in2_pool,
            tc.tile_pool(name="in1", bufs=3) as in1_pool,
            tc.tile_pool(name="ps", bufs=2, space="PSUM") as ps_pool,
            tc.tile_pool(name="ob", bufs=3) as ob_pool,
        ):
            loop = tc.For_i(0, reps, 1) if reps > 1 else contextlib.nullcontext()
            with loop:
                d2_sb = in2_pool.tile([128, 2, HP, WP], fp16, tag="d2sb")
                for ya, yb in D2_SLABS:
                    nc.sync.dma_start(
                        out=d2_sb[:, :, ya:yb, :], in_=d2r[:, :, ya:yb, :]
                    )
                for by in range(NBY):
                    d1_sb = in1_pool.tile([128, 2, NBX, 128], fp16, tag="d1sb")
                    nc.gpsimd.dma_start(out=d1_sb[:], in_=d1r[:, :, by])
                    ob = ob_pool.tile([128, NQUAD, 4, NMOV], fp16, tag="ob")
                    y0 = by * GB
                    for qb in range(NQUAD):
                        ps4 = ps_pool.tile([128, 4, 512], fp32)
                        for qq in range(4):
                            bx = qb * 4 + qq
                            x0 = bx * BB
                            for k in range(2):
                                lhsT = d1_sb[:, k, bx, :]
                                rhs = d2_sb[:, k, y0 : y0 + TT, x0 : x0 + UU]
                                nc.tensor.matmul(
                                    ps4[:, qq, 0:NMOV],
                                    lhsT,
                                    rhs,
                                    start=(k == 0),
                                    stop=(k == 1),
                                )
                        if qb % 2 == 0:
                            nc.vector.tensor_scalar_mul(
                                ob[:, qb], ps4[:, :, 0:NMOV], 1.0 / C
                            )
                        else:
                            nc.scalar.mul(ob[:, qb], ps4[:, :, 0:NMOV], 1.0 / C)
                    nc.scalar.dma_start(out=bands[by], in_=ob[:])

    nc.compile()
    return nc


def _get_nc(reps=1):
    key = ("nc", reps)
    if key not in _CACHE:
        _CACHE[key] = _build_bass(reps)
    return _CACHE[key]


def _gather_index():
    if "idx" not in _CACHE:
        g = np.arange(GB)[:, None, None, None]
        j = np.arange(BB)[None, :, None, None]
        dyp = np.arange(NG)[None, None, :, None]
        dxp = np.arange(NG)[None, None, None, :]
        _CACHE["idx"] = ((g + dyp) * UU + (j + dxp)).reshape(128, NG, NG)
    return _CACHE["idx"]


def _extract(bands_arr):
    """[NBY, 128, NBX, NMOV] fp16 bands -> [Q, H, W] fp32 output."""
    arr = np.asarray(bands_arr)
    idx = _gather_index()  # [128(m), 9, 9]
    m = np.arange(128)[:, None, None]
    # advanced indexing dims (1, 3): result [128, 9, 9, NBY, NBX]
    sub = arr[:, m, :, idx]
    # [m=(g,j), dyp, dxp, by, bx] -> [dyp, dxp, by, g, bx, j]
    sub = sub.reshape(GB, BB, NG, NG, NBY, NBX)
    out = sub.transpose(2, 3, 4, 0, 5, 1)
    return out.reshape(Q, H, W).astype(np.float32)


def prepare_inputs(data1, data2):
    d1h = np.asarray(data1, dtype=np.float16)
    d1t = (
        d1h.reshape(N_CORES, C, NBY, GB, NBX, BB)
        .transpose(0, 1, 2, 4, 3, 5)
        .reshape(N_CORES, C, NBY, NBX, GB * BB)
    )
    d2h = np.pad(
        np.asarray(data2, dtype=np.float16),
        ((0, 0), (0, 0), (PAD, PAD), (PAD, PAD)),
    )
    return [
        {
            "d1b": np.ascontiguousarray(d1t[i]),
            "d2b": np.ascontiguousarray(d2h[i]),
        }
        for i in range(N_CORES)
    ]
def _get_runner(reps=1):
    rkey = ("runner", reps)
    if rkey in _CACHE:
        return _CACHE[rkey]

    import jax
    from jax.sharding import Mesh, PartitionSpec
    from jax.experimental.shard_map import shard_map
    import concourse.mybir as mybir
    from concourse import bass2jax

    bass2jax.install_neuronx_cc_hook()
    nc = _get_nc(reps)

    partition_name = nc.partition_id_tensor.name if nc.partition_id_tensor else None
    in_names, out_names, out_avals = [], [], []
    for alloc in nc.m.functions[0].allocations:
        if not isinstance(alloc, mybir.MemoryLocationSet):
            continue
        name = alloc.memorylocations[0].name
        if alloc.kind == "ExternalInput":
            if name != partition_name:
                in_names.append(name)
        elif alloc.kind == "ExternalOutput":
            out_names.append(name)
            out_avals.append(
                jax.core.ShapedArray(
                    tuple(alloc.tensor_shape), mybir.dt.np(alloc.dtype)
                )
            )
    n_params = len(in_names)
    all_in_names = in_names + out_names
    if partition_name is not None:
        all_in_names = all_in_names + [partition_name]

    def _body(*args):
        operands = list(args)
        if partition_name is not None:
            operands.append(bass2jax.partition_id_tensor())
        outs = bass2jax._bass_exec_p.bind(
            *operands,
            out_avals=tuple(out_avals),
            in_names=tuple(all_in_names),
            out_names=tuple(out_names),
            lowering_input_output_aliases=(),
            sim_require_finite=True,
            sim_require_nnan=True,
            nc=nc,
        )
        return tuple(outs)

    devices = jax.devices()[:N_CORES]
    mesh = Mesh(np.asarray(devices), ("core",))
    n_outs = len(out_names)
    sharded = jax.jit(
        shard_map(
            _body,
            mesh=mesh,
            in_specs=(PartitionSpec("core"),) * (n_params + n_outs),
            out_specs=(PartitionSpec("core"),) * n_outs,
            check_rep=False,
        ),
        keep_unused=True,
    )
    runner = {
        "fn": sharded,
        "in_names": in_names,
        "out_names": out_names,
        "out_avals": out_avals,
        "mesh": mesh,
    }
    _CACHE[rkey] = runner
    return runner


def run_hw(in_maps):
    r = _get_runner()
    concat_in = [
        np.concatenate([m[name] for m in in_maps], axis=0) for name in r["in_names"]
    ]
    concat_zeros = [
        np.zeros((N_CORES * a.shape[0], *a.shape[1:]), a.dtype)
        for a in r["out_avals"]
    ]
    out_arrs = r["fn"](*concat_in, *concat_zeros)
    return [
        {
            name: np.asarray(out_arrs[i]).reshape(
                N_CORES, *r["out_avals"][i].shape
            )[c]
            for i, name in enumerate(r["out_names"])
        }
        for c in range(N_CORES)
    ]


def kernel(data1, data2):
    in_maps = prepare_inputs(data1, data2)
    results = run_hw(in_maps)
    out = np.stack([_extract(r["bands"]) for r in results])
    return out.astype(np.float32)



# revision 20
# speedup vs baseline: 1.7027x; 1.7027x over previous
"""Correlation kernel v6: fp8e3 moving operand + by-row-contiguous d1.

Same 16x8-block halo-matmul structure as v4 (stationary d1 block [128c x
128pos], moving 24x16 halo, K=256 as two 128-chunks accumulated in PSUM,
quad-batched PSUM evacuation alternating DVE/ACT, full-halo bands shipped
to host which does the 81-of-384 diagonal gather).  Two changes vs v4:

1. d1 DRAM layout is by-row contiguous ([NBY, C, NBX*128]) so each by-row
   load is 256 descriptors of 5 KB instead of 5120 descriptors of 256 B.
2. d2 ships as float8_e3m4 and streams into the PE as the moving operand
   (TRN2 matmul accepts mixed fp16 x fp8), halving the biggest input
   tensor (8.9 -> 4.5 MB).  d1 stays fp16 so the quantization error is
   one-sided: rel err 1.285% on the real seed vs the 2e-2 gate.

Measured (reps 512/2048 min-differencing, which suppresses the +-20 ms
axon dispatch noise): 78.9 us/iter vs v4's true 97.7 us.  Rejected via
HW ablation: strided/transposed evac destination APs (+110 us/iter) and
device-side ty-gather output DMAs with 512 B runs (87 us total) -- the
plain 2 MB/by-row output DMA with 15 KB descriptors wins.
"""

import numpy as np

C, H, W = 256, 96, 160
PAD = 4
NG = 9
Q = NG * NG
GB, BB = 16, 8
TT, UU = GB + 2 * PAD, BB + 2 * PAD  # 24 x 16
NMOV = TT * UU  # 384
NBY, NBX = H // GB, W // BB  # 6 x 20
HP, WP = H + 2 * PAD, W + 2 * PAD
NQUAD = NBX // 4  # 5 quads per by-row
N_CORES = 8

D2_FP8 = True  # ship d2 as float8_e3m4 (moving operand); d1 stays fp16

# Output mode.  Evacuation always writes the contiguous v4 layout
# ob4[128, NQUAD, 4, NMOV] (strided evac destinations measured 3x slower
# on HW).  "full" ships the whole halo (one 2 MB DMA per by-row);
# "gather" ships, per 64-partition octet h, only halo rows
# [8h, 8h+16) -- 512 B runs, 2/3 of the bytes of "full".
OUT_MODE = "full"
OUT_ROWS = 16
OUT_ALT_RING = False  # scalar-ring DMAs stall ACT's FIFO ahead of evacs

# Ablation mode for bottleneck hunting (perf experiments only):
#   "full" = normal, "noout" = skip output DMAs, "nopost" = also skip PSUM
#   evacuation, "dmaonly" = input DMAs only (no matmuls).
ABLATE = "full"

D2_SLABS = [(0, 24)] + [(24 + 16 * i, 40 + 16 * i) for i in range(NBY - 1)]

_CACHE = {}


def _np_f8():
    import ml_dtypes

    return ml_dtypes.float8_e3m4


def _build_bass(reps=1):
    import contextlib

    import concourse.bass as bass  # noqa: F401
    import concourse.mybir as mybir
    import concourse.tile as tile
    from concourse import bacc

    fp16 = mybir.dt.float16
    fp32 = mybir.dt.float32
    d2dt = mybir.dt.float8e3 if D2_FP8 else mybir.dt.float16

    nc = bacc.Bacc("TRN2", target_bir_lowering=False, debug=False)

    d1b = nc.dram_tensor(
        "d1b", [NBY, C, NBX * 128], fp16, kind="ExternalInput"
    ).ap()
    d2b = nc.dram_tensor("d2b", [C, HP, WP], d2dt, kind="ExternalInput").ap()
    if OUT_MODE == "gather":
        bands_shape = [NBY, 2, 64, NQUAD, 4, OUT_ROWS, UU]
    else:
        bands_shape = [NBY, 128, NQUAD, 4, NMOV]
    bands = nc.dram_tensor(
        "bands", bands_shape, fp16, kind="ExternalOutput"
    ).ap()

    d1r = d1b.rearrange("by (k p) f -> p k by f", p=128)
    d2r = d2b.rearrange("(k p) y x -> p k y x", p=128)

    with tile.TileContext(nc) as tc:
        with (
            tc.tile_pool(name="in2", bufs=1) as in2_pool,
            tc.tile_pool(name="in1", bufs=3) as in1_pool,
            tc.tile_pool(name="ps", bufs=2, space="PSUM") as ps_pool,
            tc.tile_pool(name="ob", bufs=3) as ob_pool,
        ):
            loop = tc.For_i(0, reps, 1) if reps > 1 else contextlib.nullcontext()
            with loop:
                d2_sb = in2_pool.tile([128, 2, HP, WP], d2dt, tag="d2sb")
                for ya, yb in D2_SLABS:
                    nc.sync.dma_start(
                        out=d2_sb[:, :, ya:yb, :], in_=d2r[:, :, ya:yb, :]
                    )
                for by in range(NBY):
                    d1_sb = in1_pool.tile([128, 2, NBX * 128], fp16, tag="d1sb")
                    nc.gpsimd.dma_start(out=d1_sb[:], in_=d1r[:, :, by])
                    ob = ob_pool.tile([128, NQUAD, 4, NMOV], fp16, tag="ob")
                    y0 = by * GB
                    for qb in range(NQUAD):
                        if ABLATE == "dmaonly":
                            break
                        ps4 = ps_pool.tile([128, 4, 512], fp32)
                        for qq in range(4):
                            bx = qb * 4 + qq
                            x0 = bx * BB
                            for k in range(2):
                                lhsT = d1_sb[:, k, bx * 128 : (bx + 1) * 128]
                                rhs = d2_sb[:, k, y0 : y0 + TT, x0 : x0 + UU]
                                nc.tensor.matmul(
                                    ps4[:, qq, 0:NMOV],
                                    lhsT,
                                    rhs,
                                    start=(k == 0),
                                    stop=(k == 1),
                                )
                        if ABLATE == "nopost":
                            continue
                        if qb % 2 == 0:
                            nc.vector.tensor_scalar_mul(
                                ob[:, qb], ps4[:, :, 0:NMOV], 1.0 / C
                            )
                        else:
                            nc.scalar.mul(
                                ob[:, qb], ps4[:, :, 0:NMOV], 1.0 / C
                            )
                    if ABLATE != "full":
                        continue
                    if OUT_MODE == "gather":
                        obr = ob.rearrange(
                            "p a b (ty tx) -> p a b ty tx", ty=TT
                        )
                        for h in range(2):
                            eng = (
                                nc.scalar
                                if (OUT_ALT_RING and h == 1)
                                else nc.sync
                            )
                            eng.dma_start(
                                out=bands[by, h],
                                in_=obr[
                                    h * 64 : (h + 1) * 64,
                                    :,
                                    :,
                                    h * 8 : h * 8 + OUT_ROWS,
                                    :,
                                ],
                            )
                    else:
                        eng = (
                            nc.scalar
                            if (OUT_ALT_RING and by % 2 == 1)
                            else nc.sync
                        )
                        eng.dma_start(out=bands[by], in_=ob[:])

    nc.compile()
    return nc


def _get_nc(reps=1):
    key = ("nc", reps)
    if key not in _CACHE:
        _CACHE[key] = _build_bass(reps)
    return _CACHE[key]


def _extract(bands_arr):
    """Device bands -> [Q, H, W] fp32 output."""
    arr = np.asarray(bands_arr)
    gg = np.arange(GB)
    jj = np.arange(BB)[None, :, None, None]
    dyy = np.arange(NG)[None, None, :, None]
    dxx = np.arange(NG)[None, None, None, :]
    tx = jj + dxx  # [1, j, 1, dx]
    if OUT_MODE == "gather":
        # arr: [NBY, 2, 64, NQUAD, 4, OUT_ROWS, UU]
        a = arr.reshape(NBY, 2, 64, NBX, OUT_ROWS, UU)
        hh = (gg // 8)[:, None, None, None]
        gl = (gg % 8)[:, None, None, None]
        pl = gl * BB + jj
        tyl = gl + dyy
        # advanced dims 1,2,4,5 (non-adjacent) -> result [g,j,dy,dx, by, bx]
        sub = a[:, hh, pl, :, tyl, tx]
    else:
        # arr: [NBY, 128, NQUAD, 4, NMOV] -> [NBY, 128, NBX, TT, UU]
        a = arr.reshape(NBY, 128, NBX, TT, UU)
        mm = gg[:, None, None, None] * BB + jj
        tyy = gg[:, None, None, None] + dyy
        # advanced dims 1,3,4 (non-adjacent) -> result [g,j,dy,dx, by, bx]
        sub = a[:, mm, :, tyy, tx]
    # [g, j, dy, dx, by, bx] -> [dy, dx, by, g, bx, j]
    out = sub.transpose(2, 3, 4, 0, 5, 1)
    return out.reshape(Q, H, W).astype(np.float32)


def prepare_inputs(data1, data2):
    d1h = np.asarray(data1, dtype=np.float16)
    d1t = (
        d1h.reshape(N_CORES, C, NBY, GB, NBX, BB)
        .transpose(0, 2, 1, 4, 3, 5)
        .reshape(N_CORES, NBY, C, NBX * GB * BB)
    )
    d2dt = _np_f8() if D2_FP8 else np.float16
    d2h = np.pad(
        np.asarray(data2, dtype=np.float32),
        ((0, 0), (0, 0), (PAD, PAD), (PAD, PAD)),
    ).astype(d2dt)
    return [
        {
            "d1b": np.ascontiguousarray(d1t[i]),
            "d2b": np.ascontiguousarray(d2h[i]),
        }
        for i in range(N_CORES)
    ]


def _get_runner(reps=1):
    rkey = ("runner", reps)
    if rkey in _CACHE:
        return _CACHE[rkey]

    import jax
    from jax.sharding import Mesh, PartitionSpec
    from jax.experimental.shard_map import shard_map
    import concourse.mybir as mybir
    from concourse import bass2jax

    bass2jax.install_neuronx_cc_hook()
    nc = _get_nc(reps)

    partition_name = nc.partition_id_tensor.name if nc.partition_id_tensor else None
    in_names, out_names, out_avals = [], [], []
    for alloc in nc.m.functions[0].allocations:
        if not isinstance(alloc, mybir.MemoryLocationSet):
            continue
        name = alloc.memorylocations[0].name
        if alloc.kind == "ExternalInput":
            if name != partition_name:
                in_names.append(name)
        elif alloc.kind == "ExternalOutput":
            out_names.append(name)
            out_avals.append(
                jax.core.ShapedArray(
                    tuple(alloc.tensor_shape), mybir.dt.np(alloc.dtype)
                )
            )
    n_params = len(in_names)
    all_in_names = in_names + out_names
    if partition_name is not None:
        all_in_names = all_in_names + [partition_name]

    def _body(*args):
        operands = list(args)
        if partition_name is not None:
            operands.append(bass2jax.partition_id_tensor())
        outs = bass2jax._bass_exec_p.bind(
            *operands,
            out_avals=tuple(out_avals),
            in_names=tuple(all_in_names),
            out_names=tuple(out_names),
            lowering_input_output_aliases=(),
            sim_require_finite=True,
            sim_require_nnan=True,
            nc=nc,
        )
        return tuple(outs)

    devices = jax.devices()[:N_CORES]
    mesh = Mesh(np.asarray(devices), ("core",))
    n_outs = len(out_names)
    sharded = jax.jit(
        shard_map(
            _body,
            mesh=mesh,
            in_specs=(PartitionSpec("core"),) * (n_params + n_outs),
            out_specs=(PartitionSpec("core"),) * n_outs,
            check_rep=False,
        ),
        keep_unused=True,
    )
    runner = {
        "fn": sharded,
        "in_names": in_names,
        "out_names": out_names,
        "out_avals": out_avals,
        "mesh": mesh,
    }
    _CACHE[rkey] = runner
    return runner


def run_hw(in_maps):
    r = _get_runner()
    concat_in = [
        np.concatenate([m[name] for m in in_maps], axis=0) for name in r["in_names"]
    ]
    concat_zeros = [
        np.zeros((N_CORES * a.shape[0], *a.shape[1:]), a.dtype)
        for a in r["out_avals"]
    ]
    out_arrs = r["fn"](*concat_in, *concat_zeros)
    return [
        {
            name: np.asarray(out_arrs[i]).reshape(
                N_CORES, *r["out_avals"][i].shape
            )[c]
            for i, name in enumerate(r["out_names"])
        }
        for c in range(N_CORES)
    ]


def kernel(data1, data2):
    in_maps = prepare_inputs(data1, data2)
    results = run_hw(in_maps)
    out = np.stack([_extract(r["bands"]) for r in results])
    return out.astype(np.float32)
